# revision 32
# baseline (speedup 1.0000x reference)
"""Baseline (v1) kernel, reconstructed for A/B clock-state testing."""
import sys
import numpy as np

for p in ("/opt/trn_rl_repo",):
    if p not in sys.path:
        sys.path.insert(0, p)

B, C, H, W = 8, 256, 64, 64
N = H * W          # 4096
CK = C // 8        # 32
NB = N // 128      # 32 key blocks
MC = N // 512      # 8 query chunks
NG = NB // 4       # 8 groups of 4 key blocks

_NC_CACHE = {}


def _build_nc():
    import concourse.mybir as mybir
    import concourse.tile as tile
    from concourse import bacc
    from concourse.bass import ds

    f32, f32r, bf16 = mybir.dt.float32, mybir.dt.float32r, mybir.dt.bfloat16
    Exp = mybir.ActivationFunctionType.Exp
    Identity = mybir.ActivationFunctionType.Identity

    nc = bacc.Bacc("TRN2", target_bir_lowering=False, debug=False)

    x_d = nc.dram_tensor("x", [C, N], f32, kind="ExternalInput").ap()
    wq_d = nc.dram_tensor("Wq", [CK, C], f32, kind="ExternalInput").ap()
    bq_d = nc.dram_tensor("bq", [CK], f32, kind="ExternalInput").ap()
    wk_d = nc.dram_tensor("Wk", [CK, C], f32, kind="ExternalInput").ap()
    bk_d = nc.dram_tensor("bk", [CK], f32, kind="ExternalInput").ap()
    wv_d = nc.dram_tensor("Wv", [C, C], f32, kind="ExternalInput").ap()
    bv_d = nc.dram_tensor("bv", [C], f32, kind="ExternalInput").ap()
    g_d = nc.dram_tensor("gamma", [1], f32, kind="ExternalInput").ap()
    y_d = nc.dram_tensor("y", [C, N], f32, kind="ExternalOutput").ap()

    x_r = x_d.rearrange("(o p) n -> p o n", p=128)   # c = o*128 + p
    y_r = y_d.rearrange("(o p) n -> p o n", p=128)

    with tile.TileContext(nc) as tc:
        with tc.tile_pool(name="const", bufs=1) as const, \
             tc.tile_pool(name="big", bufs=1) as big, \
             tc.tile_pool(name="work", bufs=4) as work, \
             tc.tile_pool(name="ptp", bufs=3) as ptp, \
             tc.tile_pool(name="ps_st", bufs=1, space="PSUM") as ps_st, \
             tc.tile_pool(name="ps_out", bufs=2, space="PSUM") as ps_out, \
             tc.tile_pool(name="ps_misc", bufs=1, space="PSUM") as ps_misc:

            from concourse.masks import make_identity
            ident = const.tile([128, 128], f32, tag="ident")
            make_identity(nc, ident[:])

            bq4 = const.tile([128, 1], f32, tag="bq4")
            bk4 = const.tile([128, 1], f32, tag="bk4")
            for j in range(4):
                nc.gpsimd.dma_start(bq4[32 * j:32 * (j + 1), :], bq_d[:, None])
                nc.gpsimd.dma_start(bk4[32 * j:32 * (j + 1), :], bk_d[:, None])
            bv2 = const.tile([128, 2], f32, tag="bv2")
            nc.gpsimd.dma_start(bv2[:], bv_d.rearrange("(o p) -> p o", p=128))
            g_col = const.tile([128, 1], f32, tag="gcol")
            nc.gpsimd.dma_start(g_col[:], g_d[None, :].to_broadcast([128, 1]))

            ones1 = const.tile([128, 1], bf16, tag="ones1")
            nc.any.memset(ones1[:], 1.0)
            ones4_raw = work.tile([4, 128], f32, tag="o4raw")
            nc.any.memset(ones4_raw[:], 1.0)
            ones4 = const.tile([4, 128], f32r, tag="ones4")
            nc.vector.tensor_copy(ones4[:], ones4_raw[:])

            gbv = const.tile([128, 2], f32, tag="gbv")
            nc.vector.tensor_scalar_mul(gbv[:], bv2[:], g_col[:])

            wq_nat = work.tile([CK, C], f32, tag="wnat")
            nc.sync.dma_start(wq_nat[:], wq_d[:])
            wk_nat = work.tile([CK, C], f32, tag="wnat")
            nc.sync.dma_start(wk_nat[:], wk_d[:])
            wqT4 = const.tile([128, 2, 128], bf16, tag="wqT4")
            wkT4 = const.tile([128, 2, 128], bf16, tag="wkT4")
            for nat, dstw in ((wq_nat, wqT4), (wk_nat, wkT4)):
                for o in range(2):
                    tp = ps_out.tile([128, CK], f32, tag="out")
                    nc.tensor.transpose(tp[:], nat[:, ds(128 * o, 128)],
                                        ident[0:CK, 0:CK])
                    for j in range(4):
                        nc.vector.tensor_copy(dstw[:, o, ds(32 * j, 32)], tp[:])

            wv_nat = work.tile([128, 2, C], f32, tag="wvnat")
            wv_n = wv_d.rearrange("(o p) c -> p o c", p=128)
            for o in range(2):
                nc.sync.dma_start(wv_nat[:, o], wv_n[:, o])
            wvT = const.tile([128, 2, C], bf16, tag="wvT")
            for o_c in range(2):
                for o_co in range(2):
                    tp = ps_out.tile([128, 128], f32, tag="out")
                    nc.tensor.transpose(tp[:], wv_nat[:, o_co, ds(128 * o_c, 128)],
                                        ident[:])
                    nc.vector.tensor_copy(wvT[:, o_c, ds(128 * o_co, 128)], tp[:])

            xs = big.tile([128, 2, N], f32, tag="xs")
            xr = big.tile([128, 2, N], bf16, tag="xr")
            q4c = [big.tile([128, 512], bf16, tag=f"q4_{i}", name=f"q4_{i}")
                   for i in range(MC)]
            k4c = [big.tile([128, 512], bf16, tag=f"k4_{i}", name=f"k4_{i}")
                   for i in range(MC)]
            vTc = [big.tile([128, 4, C], bf16, tag=f"vT_{i}", name=f"vT_{i}")
                   for i in range(MC)]
            # all x-chunk loads issued up front on the sync ring (the scalar
            # ring measures ~3-5us slower for these; gpsimd SWDGE is slower
            # still for large 2D patterns)
            for mc in range(MC):
                ms = ds(512 * mc, 512)
                nc.sync.dma_start(xs[:, :, ms], x_r[:, :, ms])

            def st_group(mc, g):
                ms_ = ds(512 * mc, 512)
                st = ps_st.tile([128, 2048], f32, tag="st", name=f"st_{mc}_{g}")
                for j in range(4):
                    nb = 4 * g + j
                    nc.tensor.matmul(st[:, ds(512 * j, 512)],
                                     k4c[nb // 4][32 * j:32 * (j + 1),
                                                  ds(128 * (nb % 4), 128)],
                                     q4c[mc][32 * j:32 * (j + 1), :],
                                     start=True, stop=True,
                                     tile_position=(32 * j, 0))
                pt = ptp.tile([128, 2048], bf16, tag="pt", name=f"pt_{mc}_{g}")
                nc.scalar.activation(pt[:], st[:], Exp)
                return pt

            # ---------- projections ----------
            # (no attention work interleaved here: chunk-0 exps queued early
            # on the strict-FIFO scalar engine would delay the later chunks'
            # q/k bias-identities, which gate the main loop's S^T groups)
            for mc in range(MC):
                ms = ds(512 * mc, 512)
                nc.vector.tensor_copy(xr[:, :, ms], xs[:, :, ms])
                for w_t, b4, dst in ((wqT4, bq4, q4c[mc]), (wkT4, bk4, k4c[mc])):
                    pp = ps_out.tile([128, 512], f32, tag="out")
                    for o in range(2):
                        nc.tensor.matmul(pp[:], w_t[:, o, :], xr[:, o, ms],
                                         start=(o == 0), stop=(o == 1))
                    nc.scalar.activation(dst[:], pp[:], Identity, bias=b4[:])
                for nb in range(4 * mc, 4 * mc + 4):
                    pv = ps_out.tile([128, C], f32, tag="out")
                    for o in range(2):
                        nc.tensor.matmul(pv[:], xr[:, o, ds(128 * nb, 128)],
                                         wvT[:, o, :], start=(o == 0), stop=(o == 1))
                    # split the PSUM->SBUF copies across DVE and the (still
                    # exp-free) scalar engine: the DVE otherwise co-paces the
                    # projection pipeline with the DMA feed
                    if nb % 2 == 0:
                        nc.vector.tensor_copy(vTc[mc][:, nb - 4 * mc, :], pv[:])
                    else:
                        nc.scalar.copy(vTc[mc][:, nb - 4 * mc, :], pv[:])
                for o in range(2):
                    nc.vector.tensor_scalar_add(xs[:, o, ms], xs[:, o, ms],
                                                gbv[:, o:o + 1])

            pending_tail = None
            pending_ssums = None
            s4c_box = {}
            pt = None
            for mc in range(MC):
                ms = ds(512 * mc, 512)
                out_ps = [ps_out.tile([128, 512], f32, tag="out", name=f"out_{mc}_{cc}")
                          for cc in range(2)]
                s_ps = ps_misc.tile([128, 512], f32, tag="sacc")
                if pt is None:
                    pt = st_group(0, 0)
                last = mc == MC - 1
                for ng in range(NG):
                    # the previous chunk's final s-sum burst + sacc copy go
                    # ahead of this slot's S^T (which head-blocks the in-order
                    # PE queue on the first exp's PSUM round-trip), filling
                    # the boundary bubble
                    if ng == 0 and pending_ssums is not None:
                        pending_ssums()
                        pending_ssums = None
                    if ng + 1 < NG:
                        next_pt = st_group(mc, ng + 1)
                    elif mc + 1 < MC:
                        next_pt = st_group(mc + 1, 0)
                    else:
                        next_pt = None

                    # on the last chunk the s-sums go first, so the final
                    # softmax reduce chain overlaps the last out-matmuls
                    # instead of trailing them
                    def ssums(g=ng, pt_=pt, s_ps_=s_ps):
                        for j in range(4):
                            nc.tensor.matmul(s_ps_[32 * j:32 * j + 1, :],
                                             ones1[:], pt_[:, ds(512 * j, 512)],
                                             start=(g == 0),
                                             stop=(g == NG - 1),
                                             tile_position=(0, 32 * j))
                    if last:
                        ssums()
                    for j in range(4):
                        nb = 4 * ng + j
                        for cc in range(2):
                            nc.tensor.matmul(out_ps[cc][:],
                                             vTc[nb // 4][:, nb % 4,
                                                          ds(128 * cc, 128)],
                                             pt[:, ds(512 * j, 512)],
                                             start=(ng == 0 and j == 0),
                                             stop=(ng == NG - 1 and j == 3))
                    if not last and ng < NG - 2:
                        ssums()
                    if not last and ng == NG - 2:
                        ssums_held = ssums
                    if not last and ng == NG - 1:
                        def pending_ssums(mc=mc, s6=ssums_held, s7=ssums,
                                          s_ps=s_ps):
                            s6()
                            s7()
                            s4c = work.tile([128, 512], f32r, tag="s4c",
                                            name=f"s4c_{mc}")
                            nc.vector.tensor_copy(s4c[:], s_ps[:])
                            s4c_box[mc] = s4c
                    if last and ng == NG - 1:
                        # kick the reduce chain while the final out-matmuls
                        # are still streaming on the PE
                        s4c = work.tile([128, 512], f32r, tag="s4c",
                                        name=f"s4c_{mc}")
                        nc.vector.tensor_copy(s4c[:], s_ps[:])
                        s4c_box[mc] = s4c
                    pt = next_pt
                    # the previous chunk's reduce/normalize chain goes after
                    # slot 0 so its DMA-gather latency never head-blocks the
                    # PE queue at the phase boundary
                    if ng == 0 and pending_tail is not None:
                        pending_tail()
                        pending_tail = None
                if not last:
                    # PSUM -> SBUF copies recycle the out banks for the next
                    # chunk; the last chunk skips them and its tail reads the
                    # accumulators straight from PSUM (no bank successor)
                    out_sb = []
                    for cc in range(2):
                        ob = work.tile([128, 512], f32, tag=f"ob{cc}",
                                       name=f"ob_{mc}_{cc}")
                        nc.vector.tensor_copy(ob[:], out_ps[cc][:])
                        out_sb.append(ob)
                else:
                    out_sb = out_ps

                def tail_a(mc=mc):
                    s4_sb = work.tile([4, 512], f32r, tag="s4")
                    nc.gpsimd.dma_start(s4_sb[:], s4c_box[mc][0:97:32, :])
                    srep_ps = ps_misc.tile([128, 512], f32, tag="srep")
                    nc.tensor.matmul(srep_ps[:], ones4[:], s4_sb[:],
                                     start=True, stop=True)
                    r_rep = work.tile([128, 512], f32, tag="rrep")
                    nc.vector.reciprocal_approx_fast(r_rep[:], srep_ps[:])
                    nc.vector.tensor_scalar_mul(r_rep[:], r_rep[:], g_col[:])
                    return r_rep

                def tail_b(r_rep, mc=mc, out_sb=out_sb, spread=False):
                    ys = ds(512 * mc, 512)
                    for cc in range(2):
                        y_sb = work.tile([128, 512], f32, tag="y")
                        t_sb = work.tile([128, 512], f32, tag="t")
                        nc.vector.tensor_mul(t_sb[:], out_sb[cc][:], r_rep[:])
                        nc.vector.tensor_add(y_sb[:], t_sb[:], xs[:, cc, ys])
                        eng = nc.gpsimd if spread and cc == 1 else nc.sync
                        eng.dma_start(y_r[:, cc, ys], y_sb[:])

                def tail():
                    tail_b(tail_a())

                pending_tail = tail
            # last chunk: its reduce chain was kicked inside the loop; the
            # stores spread over two DMA rings to halve the final drain
            tail_b(tail_a(), spread=True)

    nc.compile()
    return nc


def kernel(x, Wq, bq, Wk, bk, Wv, bv, gamma):
    from concourse import bass_utils

    if "nc" not in _NC_CACHE:
        _NC_CACHE["nc"] = _build_nc()
    nc = _NC_CACHE["nc"]

    x = np.ascontiguousarray(np.asarray(x, dtype=np.float32))
    shared = {
        "Wq": np.ascontiguousarray(np.asarray(Wq, dtype=np.float32)),
        "bq": np.ascontiguousarray(np.asarray(bq, dtype=np.float32)),
        "Wk": np.ascontiguousarray(np.asarray(Wk, dtype=np.float32)),
        "bk": np.ascontiguousarray(np.asarray(bk, dtype=np.float32)),
        "Wv": np.ascontiguousarray(np.asarray(Wv, dtype=np.float32)),
        "bv": np.ascontiguousarray(np.asarray(bv, dtype=np.float32)),
        "gamma": np.ascontiguousarray(np.asarray(gamma, dtype=np.float32)),
    }
    in_maps = [dict(shared, x=np.ascontiguousarray(x[i].reshape(C, N)))
               for i in range(B)]

    res = bass_utils.run_bass_kernel_spmd(nc, in_maps, core_ids=list(range(B)))
    y = np.stack([res.results[i]["y"] for i in range(B)], axis=0)
    return y.reshape(B, C, H, W).astype(np.float32)
